# revision 16
# baseline (speedup 1.0000x reference)
"""AGCN (ChebConv-K3 + BN + graph-max-pool) x5 + global_add_pool + MLP on 8 TRN2 cores.

Strategy:
  - Nodes sharded 12500/core (dest-sharded, degree-sorted within core, 44 pad rows -> 12544).
  - Node features live in a replicated DRAM table [100356, 64] f32 (rows 256B), rebuilt
    by AllGather after each sparse step.
  - Sparse ops (2x lhat scatter-sum + 1x segment-max per iteration) are done as
    ELL-format dma_gather (int16 indices, 2 address banks, 4 SWDGE queues) followed by
    a free-axis tensor_reduce per (tile, bank) and a per-partition scale.
  - ChebConv weights are folded host-side: t = Tx0@(W0-W2) + Tx1@W1 + lhat(Tx1)@(2*W2) + b.
  - Edge weights are separable: norm = -dinv[row]*dinv[col], so gather tables are
    pre-scaled by dinv and results scaled by -dinv[row].
  - BatchNorm is computed feature-major via matmul transposes; its affine is pushed
    past the segment-max (valid since scale > 0).
  - global_add_pool = one-hot matmul accumulation; tiny MLP at the end; AllReduce for
    BN stats and pooled graph features.
"""

import os
import numpy as np

# ---------------------------------------------------------------- constants
N_NODES = 100000
N_EDGES = 1600000
D = 48
E64 = 64            # table row payload (48 used + 16 pad) = 256B
G = 64              # graphs
H = 128
O = 12
K_CHEB = 3
N_ITERS = 5
BN_EPS = 1e-5

NCORES = 8
PER_CORE = 12500
ROWS = 12544        # per-core table rows (98 * 128)
TILES = ROWS // 128  # 98
TROWS = NCORES * ROWS   # 100352 table rows (= AG output exactly)
BASES = (0, 32768, 65536, 98304)   # idx must be non-negative int16 (<=32767)
NBANKS = 4
DUM_CORE = (1, 3, 6, 7)            # per-bank dummy rows live in these cores' pads
# dummy rows live in pad rows (locals 12540..12543)
DUM_LOCAL_Z = 12540  # two zero rows, then two -BIG rows
DUM_LOCAL_N = 12542
NEG_BIG = -3.0e38
NEG_THRESH = -1.0e37
COL_BUDGET = 112    # max staging cols per chunk
WIN = 8             # gather-call window (cols): 8*128 = 1024 idxs
NQ = 4              # SWDGE queues

_ZROWS = [c * ROWS + DUM_LOCAL_Z for c in DUM_CORE]
_NROWS = [c * ROWS + DUM_LOCAL_N for c in DUM_CORE]
for _b in range(NBANKS):
    assert BASES[_b] <= _ZROWS[_b] and _ZROWS[_b] - BASES[_b] <= 32764
    _hi = BASES[_b + 1] if _b + 1 < NBANKS else TROWS
    assert _ZROWS[_b] < _hi and _NROWS[_b] + 1 < _hi


# ---------------------------------------------------------------- host preprocessing
def _preprocess(x, edge_index, batch):
    x = np.asarray(x, np.float32)
    row = np.asarray(edge_index[0], np.int64)
    col = np.asarray(edge_index[1], np.int64)
    batch = np.asarray(batch, np.int64)
    N = N_NODES

    deg = np.bincount(row, minlength=N).astype(np.int64)
    dinv = np.where(deg > 0, 1.0 / np.sqrt(np.maximum(deg, 1)), 0.0).astype(np.float32)

    # per-core degree-desc permutation
    core_of = np.minimum(np.arange(N) // PER_CORE, NCORES - 1)
    pos_in_core = np.empty(N, np.int64)
    for c in range(NCORES):
        nodes = np.arange(c * PER_CORE, (c + 1) * PER_CORE)
        order = np.argsort(-deg[nodes], kind="stable")
        pos_in_core[nodes[order]] = np.arange(PER_CORE)
    trow = core_of * ROWS + pos_in_core  # table row of each original node

    # ---- edge lists per (core, tile, partition, bank)
    dest_core = core_of[row]
    dest_pos = pos_in_core[row]
    dest_tile = dest_pos // 128
    dest_part = dest_pos % 128
    src_trow = trow[col]
    src_bank = np.searchsorted(np.asarray(BASES[1:]), src_trow, side="right").astype(np.int64)

    # sort edges by (core, tile, part, bank) for grouped extraction
    key = ((dest_core * TILES + dest_tile) * 128 + dest_part) * NBANKS + src_bank
    eorder = np.argsort(key, kind="stable")
    key_s = key[eorder]
    src_s = src_trow[eorder]

    ngroups = NCORES * TILES * 128 * NBANKS
    counts = np.bincount(key_s, minlength=ngroups)
    starts = np.concatenate([[0], np.cumsum(counts)[:-1]])
    counts4 = counts.reshape(NCORES, TILES, 128, NBANKS)
    starts4 = starts.reshape(NCORES, TILES, 128, NBANKS)

    # homogenized K per (tile, bank): max over cores and partitions
    Ktile = counts4.max(axis=(0, 2)).astype(np.int64)  # [TILES, NBANKS]

    Ksum = Ktile.sum(axis=1)
    # chunking by column budget
    chunks = []  # list of (tile_start, ntiles)
    t0 = 0
    while t0 < TILES:
        cc, nt = 0, 0
        while t0 + nt < TILES and nt < 8:
            w = int(Ksum[t0 + nt])
            if nt > 0 and cc + w > COL_BUDGET:
                break
            cc += w
            nt += 1
        chunks.append((t0, nt))
        t0 += nt

    # dummy locals per bank per variant
    dz = [np.int64(_ZROWS[b] - BASES[b]) for b in range(NBANKS)]
    dn = [np.int64(_NROWS[b] - BASES[b]) for b in range(NBANKS)]

    # build per-core slot matrices + wrapped idx arrays
    meta_chunks = []   # per chunk: dict with layout info
    idx_sum = [[] for _ in range(NCORES)]
    idx_max = [[] for _ in range(NCORES)]
    wofs = 0
    for (ts, nt) in chunks:
        ccb = [int(Ktile[ts:ts + nt, b].sum()) for b in range(NBANKS)]
        bofs = np.concatenate([[0], np.cumsum(ccb)]).astype(np.int64)
        cc = int(bofs[-1])
        # per-core chunk slot matrices [128, cc]
        mats_s = []
        mats_m = []
        for c in range(NCORES):
            m_s = np.empty((128, cc), np.int64)
            m_m = np.empty((128, cc), np.int64)
            off = [int(x) for x in bofs[:NBANKS]]
            for j in range(nt):
                t = ts + j
                for b in range(NBANKS):
                    Kb = int(Ktile[t, b])
                    if Kb == 0:
                        continue
                    o = off[b]
                    base = BASES[b]
                    sub_s = np.full((128, Kb), dz[b], np.int64)
                    sub_m = np.full((128, Kb), dn[b], np.int64)
                    cnt = counts4[c, t, :, b]
                    st = starts4[c, t, :, b]
                    for p in range(128):
                        k = int(cnt[p])
                        if k:
                            vals = src_s[st[p]:st[p] + k] - base
                            sub_s[p, :k] = vals
                            sub_m[p, :k] = vals
                    m_s[:, o:o + Kb] = sub_s
                    m_m[:, o:o + Kb] = sub_m
                    off[b] += Kb
            mats_s.append(m_s)
            mats_m.append(m_m)
        # windows: per bank region, consecutive WIN-col calls
        wins = []  # (colstart, width, bank)
        for b in range(NBANKS):
            a, hi = int(bofs[b]), int(bofs[b + 1])
            while a < hi:
                w = min(WIN, hi - a)
                wins.append((a, w, b))
                a += w
        # wrapped idx blocks per core
        for c in range(NCORES):
            for (a, w, b) in wins:
                for arrs, mats in ((idx_sum, mats_s), (idx_max, mats_m)):
                    m = mats[c][:, a:a + w]                    # [128, w]
                    flat = m.T.reshape(-1)                     # position i = k*128+p
                    wrapped = flat.reshape(w * 8, 16).T        # [16, 8w]
                    arrs[c].append(np.tile(wrapped, (8, 1)))   # [128, 8w]
        meta_chunks.append(dict(ts=ts, nt=nt, cc=cc, bofs=[int(x) for x in bofs],
                                wins=wins, wofs=wofs,
                                K=[[int(Ktile[ts + j, b]) for b in range(NBANKS)]
                                   for j in range(nt)]))
        wofs += 8 * cc

    idx_sum = [np.ascontiguousarray(np.concatenate(a, axis=1), np.int16) for a in idx_sum]
    idx_max = [np.ascontiguousarray(np.concatenate(a, axis=1), np.int16) for a in idx_max]
    WTOT = idx_sum[0].shape[1]

    # ---- per-core dense arrays in (partition, tile) layout
    def core_layout(vec, fill=0.0):
        out = np.full((NCORES, 128, TILES), fill, np.float32)
        for c in range(NCORES):
            nodes = np.arange(c * PER_CORE, (c + 1) * PER_CORE)
            pos = pos_in_core[nodes]
            out[c, pos % 128, pos // 128] = vec[nodes]
        return out

    mdinv = core_layout(-dinv)
    dinv2m = core_layout(-dinv * dinv)
    pdinv = core_layout(dinv)

    xloc = np.zeros((NCORES, 128, TILES, D), np.float32)
    bc = np.zeros((NCORES, 128, TILES, G), np.float32)
    for c in range(NCORES):
        nodes = np.arange(c * PER_CORE, (c + 1) * PER_CORE)
        pos = pos_in_core[nodes]
        xloc[c, pos % 128, pos // 128, :] = x[nodes]
        bc[c, pos % 128, pos // 128, batch[nodes]] = 1.0

    # ---- initial XS table (dinv * x), with dummies
    xs_init = np.zeros((TROWS, E64), np.float32)
    xs_init[trow, :D] = x * dinv[:, None]
    for b in range(NBANKS):
        xs_init[_NROWS[b]:_NROWS[b] + 2, :] = NEG_BIG
        xs_init[_ZROWS[b]:_ZROWS[b] + 2, :] = 0.0

    meta = dict(chunks=meta_chunks, WTOT=WTOT)
    percore = dict(idx_sum=idx_sum, idx_max=idx_max, mdinv=mdinv, dinv2m=dinv2m,
                   pdinv=pdinv, xloc=xloc, bc=bc, xs_init=xs_init)
    return meta, percore


# ---------------------------------------------------------------- program builder
def _build(meta):
    from concourse import bacc, bass, mybir, tile, library_config
    from concourse.masks import make_identity

    fp32 = mybir.dt.float32
    Alu = mybir.AluOpType
    Act = mybir.ActivationFunctionType

    nc = bacc.Bacc(num_devices=NCORES, num_swdge_queues=NQ)
    _ = bass  # keep import
    WTOT = meta["WTOT"]
    chunks = meta["chunks"]

    # ---------------- I/O
    t_xs_init = nc.dram_tensor("xs_init", [TROWS, E64], fp32, kind="ExternalInput")
    t_xloc = nc.dram_tensor("xloc", [128, TILES, D], fp32, kind="ExternalInput")
    t_mdinv = nc.dram_tensor("mdinv", [128, TILES], fp32, kind="ExternalInput")
    t_dinv2m = nc.dram_tensor("dinv2m", [128, TILES], fp32, kind="ExternalInput")
    t_pdinv = nc.dram_tensor("pdinv", [128, TILES], fp32, kind="ExternalInput")
    t_idx_sum = nc.dram_tensor("idx_sum", [128, WTOT], mybir.dt.int16, kind="ExternalInput")
    t_idx_max = nc.dram_tensor("idx_max", [128, WTOT], mybir.dt.int16, kind="ExternalInput")
    t_bc = nc.dram_tensor("bc", [128, TILES, G], fp32, kind="ExternalInput")
    t_w0p = nc.dram_tensor("w0p", [D, D], fp32, kind="ExternalInput")
    t_w1c = nc.dram_tensor("w1c", [D, D], fp32, kind="ExternalInput")
    t_w2x2 = nc.dram_tensor("w2x2", [D, D], fp32, kind="ExternalInput")
    t_bias = nc.dram_tensor("bias48", [D, 1], fp32, kind="ExternalInput")
    t_gamma = nc.dram_tensor("gamma_fm", [D, 1], fp32, kind="ExternalInput")
    t_beta = nc.dram_tensor("beta_fm", [D, 1], fp32, kind="ExternalInput")
    t_w1b1 = nc.dram_tensor("w1b1", [D + 1, H], fp32, kind="ExternalInput")
    t_w2 = nc.dram_tensor("w2m", [H, O], fp32, kind="ExternalInput")
    t_b2 = nc.dram_tensor("b2m", [1, O], fp32, kind="ExternalInput")
    t_out = nc.dram_tensor("out", [G, O], fp32, kind="ExternalOutput")
    dbg = os.environ.get("DEBUG_DUMPS")
    if dbg:
        t_dbg_tx1 = nc.dram_tensor("dbg_tx1", [128, TILES, D], fp32, kind="ExternalOutput")
        t_dbg_tt = nc.dram_tensor("dbg_tt", [ROWS, E64], fp32, kind="ExternalOutput")
        t_dbg_out = nc.dram_tensor("dbg_outl", [128, TILES, D], fp32, kind="ExternalOutput")
        t_dbg_st = nc.dram_tensor("dbg_st", [D, 2], fp32, kind="ExternalOutput")

    # ---------------- internal DRAM
    groups = [list(range(NCORES))]
    tables = {}
    agins = {}
    for name in ("xs", "xs1", "tt"):
        tables[name] = nc.dram_tensor(f"tab_{name}", [TROWS, E64], fp32, addr_space="Shared")
        agins[name] = nc.dram_tensor(f"agin_{name}", [ROWS, E64], fp32)
    ar_in = nc.dram_tensor("ar_in", [D, 2], fp32)
    ar_out = nc.dram_tensor("ar_out", [D, 2], fp32, addr_space="Shared")
    gar_in = nc.dram_tensor("gar_in", [G, D], fp32)
    gar_out = nc.dram_tensor("gar_out", [G, D], fp32, addr_space="Shared")

    CCMAX = max(ch["cc"] for ch in chunks)
    NTMAX = max(ch["nt"] for ch in chunks)

    qctr = [0]

    def nextq():
        qctr[0] = (qctr[0] + 1) % NQ
        return qctr[0]

    with tile.TileContext(nc) as tc:
        nc.gpsimd.load_library(library_config.mlp)
        with (
            tc.tile_pool(name="persist", bufs=1) as pp,
            tc.tile_pool(name="stage", bufs=2) as stp,
            tc.tile_pool(name="idxp", bufs=2) as idxp,
            tc.tile_pool(name="small", bufs=4) as smp,
            tc.tile_pool(name="epil", bufs=2) as epp,
            tc.tile_pool(name="xtp", bufs=3) as xtp,
            tc.tile_pool(name="psA", bufs=2, space="PSUM") as psA,
            tc.tile_pool(name="psB", bufs=2, space="PSUM") as psB,
            tc.tile_pool(name="psC", bufs=2, space="PSUM") as psC,
            tc.tile_pool(name="psD", bufs=1, space="PSUM") as psD,
        ):
            # ------ persistent SBUF state
            OUT_L = pp.tile([128, TILES, D], fp32)
            TX1_L = pp.tile([128, TILES, D], fp32)
            mdinv_t = pp.tile([128, TILES], fp32)
            dinv2m_t = pp.tile([128, TILES], fp32)
            pdinv_t = pp.tile([128, TILES], fp32)
            bc_t = pp.tile([128, TILES, G], fp32)
            w0p_t = pp.tile([D, D], fp32)
            w1c_t = pp.tile([D, D], fp32)
            w2x2_t = pp.tile([D, D], fp32)
            bias_t = pp.tile([D, 1], fp32)
            gamma_t = pp.tile([D, 1], fp32)
            beta_t = pp.tile([D, 1], fp32)
            w1b1_t = pp.tile([D + 1, H], fp32)
            w2_t = pp.tile([H, O], fp32)
            b2_t = pp.tile([1, O], fp32)
            ident = pp.tile([128, 128], fp32)
            ones_r = pp.tile([1, 128], fp32)
            zeros48 = pp.tile([128, D], fp32)
            dumz = pp.tile([2, E64], fp32)
            dumn = pp.tile([2, E64], fp32)
            ssum = pp.tile([D, TILES], fp32)
            ssq = pp.tile([D, TILES], fp32)
            scaleB = pp.tile([128, D], fp32)
            shiftB = pp.tile([128, D], fp32)

            make_identity(nc, ident[:])
            nc.vector.memset(ones_r[:], 1.0)
            nc.vector.memset(zeros48[:], 0.0)
            nc.vector.memset(dumz[:], 0.0)
            nc.vector.memset(dumn[:], NEG_BIG)

            nc.sync.dma_start(out=OUT_L[:], in_=t_xloc[:])
            nc.sync.dma_start(out=mdinv_t[:], in_=t_mdinv[:])
            nc.sync.dma_start(out=dinv2m_t[:], in_=t_dinv2m[:])
            nc.sync.dma_start(out=pdinv_t[:], in_=t_pdinv[:])
            nc.sync.dma_start(out=bc_t[:], in_=t_bc[:])
            nc.sync.dma_start(out=w0p_t[:], in_=t_w0p[:])
            nc.sync.dma_start(out=w1c_t[:], in_=t_w1c[:])
            nc.sync.dma_start(out=w2x2_t[:], in_=t_w2x2[:])
            nc.sync.dma_start(out=bias_t[:], in_=t_bias[:])
            nc.sync.dma_start(out=gamma_t[:], in_=t_gamma[:])
            nc.sync.dma_start(out=beta_t[:], in_=t_beta[:])
            nc.sync.dma_start(out=w1b1_t[:], in_=t_w1b1[:])
            nc.sync.dma_start(out=w2_t[:], in_=t_w2[:])
            nc.sync.dma_start(out=b2_t[:], in_=t_b2[:])
            # initial XS table
            nc.sync.dma_start(out=tables["xs"][:], in_=t_xs_init[:])

            bank_slice = {b: (BASES[b], BASES[b] + 2) for b in range(NBANKS)}

            def gather_chunk(ch, table, idx_dram, redop, per_tile_fn):
                """Gather one chunk from `table`, reduce per (tile,bank), call
                per_tile_fn(u_ap, global_tile, j) for each tile; returns epil tile."""
                cc, wins, wofs = ch["cc"], ch["wins"], ch["wofs"]
                nt, ts = ch["nt"], ch["ts"]
                idx_t = idxp.tile([128, 8 * max(CCMAX, 1)], mybir.dt.int16, tag="idx")
                stage = stp.tile([128, max(CCMAX, 1), E64], fp32, tag="stage")
                if cc:
                    nc.sync.dma_start(out=idx_t[:, : 8 * cc],
                                      in_=idx_dram[:, wofs:wofs + 8 * cc])
                if os.environ.get("NOGATHER"):
                    nc.vector.memset(stage[:, :cc, :].rearrange("p c e -> p (c e)"), 0.0)
                else:
                    for (a, w, b) in wins:
                        lo, hi = bank_slice[b]
                        nc.gpsimd.dma_gather(
                            stage[:, a:a + w, :],
                            table[lo:hi, :],
                            idx_t[:, 8 * a: 8 * (a + w)],
                            w * 128, w * 128, E64,
                            queue_num=nextq(),
                        )
                offs = list(ch["bofs"][:NBANKS])
                treemode = not os.environ.get("STRIDED_REDUCE")
                for j in range(nt):
                    u = None
                    for b in range(NBANKS):
                        Kb = ch["K"][j][b]
                        if Kb == 0:
                            continue
                        a = offs[b]
                        if treemode:
                            # contiguous in-place folds on stage
                            w = Kb
                            while w > 1:
                                h = w // 2
                                nc.vector.tensor_tensor(
                                    out=stage[:, a:a + h, :],
                                    in0=stage[:, a:a + h, :],
                                    in1=stage[:, a + w - h:a + w, :], op=redop)
                                w -= h
                            if u is None:
                                u = smp.tile([128, E64], fp32, tag="u")
                                nc.vector.tensor_copy(out=u[:], in_=stage[:, a, :])
                            else:
                                nc.vector.tensor_tensor(out=u[:], in0=u[:],
                                                        in1=stage[:, a, :], op=redop)
                        else:
                            if u is None:
                                u = smp.tile([128, E64], fp32, tag="u")
                                nc.vector.tensor_reduce(
                                    out=u[:],
                                    in_=stage[:, a:a + Kb, :].rearrange("p k e -> p e k"),
                                    axis=mybir.AxisListType.X, op=redop)
                            else:
                                u1 = smp.tile([128, E64], fp32, tag="u1")
                                nc.vector.tensor_reduce(
                                    out=u1[:],
                                    in_=stage[:, a:a + Kb, :].rearrange("p k e -> p e k"),
                                    axis=mybir.AxisListType.X, op=redop)
                                nc.vector.tensor_tensor(out=u[:], in0=u[:], in1=u1[:], op=redop)
                        offs[b] += Kb
                    if u is None:
                        u = smp.tile([128, E64], fp32, tag="u")
                        nc.vector.memset(u[:], 0.0 if redop == Alu.add else NEG_BIG)
                    per_tile_fn(u, ts + j, j)

            def agin_view(agin, ts, nt):
                return agin[:].rearrange("(t p) e -> p t e", t=TILES)[:, ts:ts + nt, :D]

            def finish_agin_and_ag(name):
                agin, table = agins[name], tables[name]
                nc.sync.dma_start(out=agin[DUM_LOCAL_Z:DUM_LOCAL_Z + 2, :], in_=dumz[:])
                nc.sync.dma_start(out=agin[DUM_LOCAL_N:DUM_LOCAL_N + 2, :], in_=dumn[:])
                nc.gpsimd.collective_compute(
                    "AllGather", Alu.bypass, replica_groups=groups,
                    ins=[agin[:]], outs=[table[:]],
                )

            # ================= iteration body =================
            for it in range(N_ITERS):
                # ---------- pass A: u = sum(XS[col]); Tx1 = -dinv*u; agin_xs1 = -dinv^2*u
                for ch in chunks:
                    nt, ts = ch["nt"], ch["ts"]
                    ep = epp.tile([128, NTMAX, D], fp32, tag="epA")

                    def fA(u, t, j, ep=ep):
                        nc.vector.tensor_scalar(
                            out=TX1_L[:, t, :], in0=u[:, :D],
                            scalar1=mdinv_t[:, t:t + 1], scalar2=None, op0=Alu.mult)
                        nc.vector.tensor_scalar(
                            out=ep[:, j, :], in0=u[:, :D],
                            scalar1=dinv2m_t[:, t:t + 1], scalar2=None, op0=Alu.mult)

                    gather_chunk(ch, tables["xs"], t_idx_sum, Alu.add, fA)
                    nc.sync.dma_start(out=agin_view(agins["xs1"], ts, nt), in_=ep[:, :nt, :])
                finish_agin_and_ag("xs1")

                # ---------- pass B: u = sum(XS1[col]); V = -dinv*u; matmuls; BN stats; agin_t
                for ci, ch in enumerate(chunks):
                    nt, ts = ch["nt"], ch["ts"]
                    ep = epp.tile([128, NTMAX, D], fp32, tag="epB")
                    vlist = []

                    def fB(u, t, j, vlist=vlist):
                        v = smp.tile([128, D], fp32, tag="v")
                        nc.vector.tensor_scalar(
                            out=v[:], in0=u[:, :D],
                            scalar1=mdinv_t[:, t:t + 1], scalar2=None, op0=Alu.mult)
                        vlist.append(v)

                    gather_chunk(ch, tables["xs1"], t_idx_sum, Alu.add, fB)
                    # matmul stage per tile
                    for j in range(nt):
                        t = ts + j
                        accT = psA.tile([D, 128], fp32, space="PSUM", tag="accT")
                        for k, (w_t, xsrc) in enumerate((
                                (w0p_t, OUT_L[:, t, :]),
                                (w1c_t, TX1_L[:, t, :]),
                                (w2x2_t, vlist[j][:]))):
                            xT_ps = psB.tile([D, 128], fp32, space="PSUM", tag="xT")
                            nc.tensor.transpose(out=xT_ps[:], in_=xsrc, identity=ident[:])
                            xT_sb = xtp.tile([D, 128], fp32, tag="xTsb")
                            nc.vector.tensor_copy(out=xT_sb[:], in_=xT_ps[:])
                            nc.tensor.matmul(out=accT[:], lhsT=w_t[:], rhs=xT_sb[:],
                                             start=(k == 0), stop=(k == 2))
                        traw = xtp.tile([D, 128], fp32, tag="traw")
                        nc.scalar.activation(out=traw[:], in_=accT[:],
                                             func=Act.Relu, bias=bias_t[:], scale=1.0)
                        valid = 128 if not (t == TILES - 1) else (PER_CORE - (TILES - 1) * 128)
                        col = t
                        nc.vector.tensor_reduce(out=ssum[:, col:col + 1],
                                                in_=traw[:, :valid],
                                                axis=mybir.AxisListType.X, op=Alu.add)
                        sq = xtp.tile([D, 128], fp32, tag="sq")
                        nc.scalar.activation(out=sq[:, :valid], in_=traw[:, :valid],
                                             func=Act.Square)
                        nc.vector.tensor_reduce(out=ssq[:, col:col + 1],
                                                in_=sq[:, :valid],
                                                axis=mybir.AxisListType.X, op=Alu.add)
                        tb_ps = psC.tile([128, D], fp32, space="PSUM", tag="tb")
                        nc.tensor.matmul(out=tb_ps[:], lhsT=traw[:], rhs=ident[:D, :D],
                                         is_transpose=True)
                        nc.vector.tensor_copy(out=ep[:, j, :], in_=tb_ps[:])
                    nc.sync.dma_start(out=agin_view(agins["tt"], ts, nt), in_=ep[:, :nt, :])
                finish_agin_and_ag("tt")

                if dbg and it == 0:
                    nc.sync.dma_start(out=t_dbg_tx1[:], in_=TX1_L[:])
                    nc.sync.dma_start(out=t_dbg_tt[:], in_=agins["tt"][:])
                # ---------- BN stats AllReduce + scale/shift
                st2 = smp.tile([D, 2], fp32, tag="st2")
                nc.vector.tensor_reduce(out=st2[:, 0:1], in_=ssum[:],
                                        axis=mybir.AxisListType.X, op=Alu.add)
                nc.vector.tensor_reduce(out=st2[:, 1:2], in_=ssq[:],
                                        axis=mybir.AxisListType.X, op=Alu.add)
                nc.sync.dma_start(out=ar_in[:], in_=st2[:])
                nc.gpsimd.collective_compute(
                    "AllReduce", Alu.add, replica_groups=groups,
                    ins=[ar_in[:]], outs=[ar_out[:]])
                stg = smp.tile([D, 2], fp32, tag="stg")
                nc.sync.dma_start(out=stg[:], in_=ar_out[:])
                mean = smp.tile([D, 1], fp32, tag="mean")
                nc.vector.tensor_scalar(out=mean[:], in0=stg[:, 0:1],
                                        scalar1=1.0 / N_NODES, scalar2=None, op0=Alu.mult)
                var = smp.tile([D, 1], fp32, tag="var")
                nc.vector.tensor_scalar(out=var[:], in0=stg[:, 1:2],
                                        scalar1=1.0 / N_NODES, scalar2=None, op0=Alu.mult)
                mm = smp.tile([D, 1], fp32, tag="mm")
                nc.vector.tensor_tensor(out=mm[:], in0=mean[:], in1=mean[:], op=Alu.mult)
                nc.vector.tensor_tensor(out=var[:], in0=var[:], in1=mm[:], op=Alu.subtract)
                nc.vector.tensor_scalar(out=var[:], in0=var[:],
                                        scalar1=float(BN_EPS), scalar2=None, op0=Alu.add)
                inv = smp.tile([D, 1], fp32, tag="inv")
                nc.vector.reciprocal(out=inv[:], in_=var[:])
                sroot = smp.tile([D, 1], fp32, tag="sroot")
                nc.scalar.activation(out=sroot[:], in_=inv[:], func=Act.Sqrt)
                scsh = smp.tile([D, 2], fp32, tag="scsh")
                nc.vector.tensor_tensor(out=scsh[:, 0:1], in0=sroot[:], in1=gamma_t[:], op=Alu.mult)
                nc.vector.tensor_tensor(out=scsh[:, 1:2], in0=mean[:], in1=scsh[:, 0:1], op=Alu.mult)
                tmpb = smp.tile([D, 1], fp32, tag="tmpb")
                nc.vector.tensor_tensor(out=tmpb[:], in0=beta_t[:], in1=scsh[:, 1:2], op=Alu.subtract)
                nc.vector.tensor_copy(out=scsh[:, 1:2], in_=tmpb[:])
                scr_ps = psD.tile([1, D], fp32, space="PSUM", tag="misc")
                nc.tensor.matmul(out=scr_ps[:], lhsT=scsh[:, 0:1], rhs=ident[:D, :D],
                                 is_transpose=True)
                scr = smp.tile([1, D], fp32, tag="scr")
                nc.vector.tensor_copy(out=scr[:], in_=scr_ps[:])
                shr_ps = psD.tile([1, D], fp32, space="PSUM", tag="misc")
                nc.tensor.matmul(out=shr_ps[:], lhsT=scsh[:, 1:2], rhs=ident[:D, :D],
                                 is_transpose=True)
                shr = smp.tile([1, D], fp32, tag="shr")
                nc.vector.tensor_copy(out=shr[:], in_=shr_ps[:])
                sb_ps = psD.tile([128, D], fp32, space="PSUM", tag="misc")
                nc.tensor.matmul(out=sb_ps[:], lhsT=ones_r[:], rhs=scr[:],
                                 start=True, stop=True)
                nc.vector.tensor_copy(out=scaleB[:], in_=sb_ps[:])
                sh_ps = psD.tile([128, D], fp32, space="PSUM", tag="misc")
                nc.tensor.matmul(out=sh_ps[:], lhsT=ones_r[:], rhs=shr[:],
                                 start=True, stop=True)
                nc.vector.tensor_copy(out=shiftB[:], in_=sh_ps[:])

                # ---------- pass C: u = max(T[col]); affine; select; OUT_L; agin_xs
                for ch in chunks:
                    nt, ts = ch["nt"], ch["ts"]
                    ep = epp.tile([128, NTMAX, D], fp32, tag="epC")

                    def fC(u, t, j, ep=ep):
                        mask = smp.tile([128, D], mybir.dt.uint8, tag="mask")
                        nc.vector.tensor_scalar(out=mask[:], in0=u[:, :D],
                                                scalar1=float(NEG_THRESH), scalar2=None,
                                                op0=Alu.is_lt)
                        nc.vector.tensor_tensor(out=OUT_L[:, t, :], in0=u[:, :D],
                                                in1=scaleB[:], op=Alu.mult)
                        nc.vector.tensor_tensor(out=OUT_L[:, t, :], in0=OUT_L[:, t, :],
                                                in1=shiftB[:], op=Alu.add)
                        nc.vector.copy_predicated(out=OUT_L[:, t, :], mask=mask[:],
                                                  data=zeros48[:])
                        nc.vector.tensor_scalar(
                            out=ep[:, j, :], in0=OUT_L[:, t, :],
                            scalar1=pdinv_t[:, t:t + 1], scalar2=None, op0=Alu.mult)

                    gather_chunk(ch, tables["tt"], t_idx_max, Alu.max, fC)
                    nc.sync.dma_start(out=agin_view(agins["xs"], ts, nt), in_=ep[:, :nt, :])
                if dbg and it == 0:
                    nc.sync.dma_start(out=t_dbg_out[:], in_=OUT_L[:])
                    nc.sync.dma_start(out=t_dbg_st[:], in_=stg[:])
                if it < N_ITERS - 1:
                    finish_agin_and_ag("xs")

            # ================= global_add_pool + MLP =================
            g_ps = psD.tile([G, D], fp32, space="PSUM", tag="misc")
            for t in range(TILES):
                nc.tensor.matmul(out=g_ps[:], lhsT=bc_t[:, t, :], rhs=OUT_L[:, t, :],
                                 start=(t == 0), stop=(t == TILES - 1))
            g_sb = smp.tile([G, D], fp32, tag="gsb")
            nc.vector.tensor_copy(out=g_sb[:], in_=g_ps[:])
            nc.sync.dma_start(out=gar_in[:], in_=g_sb[:])
            nc.gpsimd.collective_compute(
                "AllReduce", Alu.add, replica_groups=groups,
                ins=[gar_in[:]], outs=[gar_out[:]])
            g2 = smp.tile([G, D], fp32, tag="g2")
            nc.sync.dma_start(out=g2[:], in_=gar_out[:])
            gT_ps = psA.tile([D, G], fp32, space="PSUM", tag="accT")
            nc.tensor.matmul(out=gT_ps[:], lhsT=g2[:], rhs=ident[:G, :G], is_transpose=True)
            gT1 = smp.tile([D + 1, G], fp32, tag="gT1")
            nc.vector.memset(gT1[:], 1.0)
            nc.vector.tensor_copy(out=gT1[:D, :], in_=gT_ps[:])
            h_ps = psD.tile([G, H], fp32, space="PSUM", tag="misc")
            nc.tensor.matmul(out=h_ps[:], lhsT=gT1[:], rhs=w1b1_t[:], start=True, stop=True)
            h_sb = smp.tile([G, H], fp32, tag="hsb")
            nc.scalar.activation(out=h_sb[:], in_=h_ps[:], func=Act.Relu)
            hT_ps = psA.tile([H, G], fp32, space="PSUM", tag="accT")
            nc.tensor.matmul(out=hT_ps[:], lhsT=h_sb[:], rhs=ident[:G, :G], is_transpose=True)
            hT_sb = smp.tile([H, G], fp32, tag="hTsb")
            nc.vector.tensor_copy(out=hT_sb[:], in_=hT_ps[:])
            o_ps = psC.tile([G, O], fp32, space="PSUM", tag="tb")
            nc.tensor.matmul(out=o_ps[:], lhsT=hT_sb[:], rhs=w2_t[:], start=True, stop=False)
            nc.tensor.matmul(out=o_ps[:], lhsT=ones_r[:, :G], rhs=b2_t[:], start=False, stop=True)
            o_sb = smp.tile([G, O], fp32, tag="osb")
            nc.vector.tensor_copy(out=o_sb[:], in_=o_ps[:])
            nc.sync.dma_start(out=t_out[:], in_=o_sb[:])

    nc.compile()
    return nc


# ---------------------------------------------------------------- runner
def _run(nc, in_maps):
    from concourse.bass_utils import run_bass_kernel_spmd
    res = run_bass_kernel_spmd(nc, in_maps, list(range(NCORES)))
    return res.results


def kernel(x, edge_index, batch, num_graphs, W, b, gamma, beta, W1, b1, W2, b2):
    x = np.asarray(x, np.float32)
    W = np.asarray(W, np.float32)
    b = np.asarray(b, np.float32)
    gamma = np.asarray(gamma, np.float32)
    beta = np.asarray(beta, np.float32)
    W1 = np.asarray(W1, np.float32)
    b1 = np.asarray(b1, np.float32)
    W2 = np.asarray(W2, np.float32)
    b2 = np.asarray(b2, np.float32)

    meta, pc = _preprocess(x, edge_index, batch)
    nc = _build(meta)

    shared = dict(
        xs_init=pc["xs_init"],
        w0p=np.ascontiguousarray(W[0] - W[2]),
        w1c=np.ascontiguousarray(W[1]),
        w2x2=np.ascontiguousarray(2.0 * W[2]),
        bias48=b.reshape(D, 1),
        gamma_fm=gamma.reshape(D, 1),
        beta_fm=beta.reshape(D, 1),
        w1b1=np.ascontiguousarray(np.vstack([W1, b1.reshape(1, H)])),
        w2m=W2,
        b2m=b2.reshape(1, O),
    )
    in_maps = []
    for c in range(NCORES):
        m = dict(shared)
        m.update(
            xloc=pc["xloc"][c],
            mdinv=pc["mdinv"][c],
            dinv2m=pc["dinv2m"][c],
            pdinv=pc["pdinv"][c],
            idx_sum=pc["idx_sum"][c],
            idx_max=pc["idx_max"][c],
            bc=pc["bc"][c],
        )
        in_maps.append(m)

    results = _run(nc, in_maps)
    if os.environ.get("DEBUG_DUMPS"):
        kernel._dbg = (results, meta, pc)
    return results[0]["out"].astype(np.float32)


if __name__ == "__main__":
    # quick selftest with subsampled edges against the jax reference
    import sys
    sys.path.insert(0, os.path.dirname(os.path.abspath(__file__)))
    import jax
    import reference

    cpu = jax.devices("cpu")[0]
    with jax.default_device(cpu):
        inputs = reference.setup_inputs()
    ne = int(os.environ.get("SELFTEST_EDGES", "0"))
    if ne:
        inputs = dict(inputs)
        inputs["edge_index"] = inputs["edge_index"][:, :ne]
    with jax.default_device(cpu):
        exp = np.asarray(reference.reference(**inputs))
    got = kernel(**{k: np.asarray(v) for k, v in inputs.items()})
    err = np.abs(got - exp).max() / (np.abs(exp).max() + 1e-9)
    print("Relative error:", err)
    print("PASS" if err < 2e-2 else "FAIL")
